# revision 2
# baseline (speedup 1.0000x reference)
"""Trainium2 Bass kernel for nn_CrossAttUnit (ragged cross-attention unit).

Math (per 64-token segment, N=262144 tokens total, H=256, D=64):
    yk = y_seg @ k            [64, 64]
    yq = yhat_seg @ q         [64, 64]
    M  = (yk @ yq.T) / 8      [64, 64]
    attn = softmax(M, axis=1) + 1e-6      (row softmax)
    W  = attn / attn.sum(axis=0)          (column normalize)
Output: [4096, 64, 64] float32.

Sharding: data-parallel over segments; core i handles tokens
[i*32768, (i+1)*32768) (512 whole segments). k, q replicated.

Per-core dataflow (tokens arrive token-major [tok, H]; every useful
contraction is over H or D, so each 128-token tile is PE-transposed to
[H, tok] first):
  tile = 128 tokens (2 segments):
    y/yhat tile --PE transpose (matmul vs identity)--> yT [256, 128] in PSUM
    copy PSUM->SBUF (split DVE/ACT)
    ykT = k.T @ yT, yqT = q.T @ yT  (PE, contract H in 2 chunks) -> [64, 128]
    seg matmul: M_seg = yk_seg @ yq_seg.T (PE, contract D=64, 4-way
    tile_position packing across a pair of tiles)
  group = 4 tiles (8 segments), M in PSUM as [128, 4, 64]
    row max (DVE) -> exp+rowsum fused (ACT, accum_out) -> 1/rowsum (DVE)
    attn' = E*rr + EPS (GPSIMD tensor_scalar)
    colsum broadcast = blockdiag_ones.T @ attn' (PE) -> [128, 4, 64]
    rc = exp(-ln(colsum)) (ACT)  ;  W = attn' * rc (GPSIMD)
"""

import os

import numpy as np

N_TOTAL = 262144
H = 256
D = 64
L = 64  # seg_len
NCORES = 8
N_LOC = N_TOTAL // NCORES  # 32768
TILE_TOK = 128
GROUP_TILES = 4
GROUP_TOK = TILE_TOK * GROUP_TILES  # 512
SCALE = 0.125  # 1/sqrt(D)
EPS = 1e-6

_CACHE = {}


def _build_program(n_loc):
    import concourse.bacc as bacc
    import concourse.tile as tile
    from concourse import mybir

    f32 = mybir.dt.float32
    FT = mybir.ActivationFunctionType
    OP = mybir.AluOpType

    nc = bacc.Bacc("TRN2", target_bir_lowering=False)

    y_d = nc.dram_tensor("y", [n_loc, H], f32, kind="ExternalInput")
    yh_d = nc.dram_tensor("yh", [n_loc, H], f32, kind="ExternalInput")
    # k/q are zero-padded to [H, 128] on the host (cols D..127 zero) so the
    # projection matmuls emit [128, tok] tiles whose upper 64 partitions are
    # zeros.  That keeps every PE matmul in the kernel a full-K=128
    # row-base-0 matmul -- sequences of partial-row (K=64) matmuls whose
    # stationary partition base changes were observed to hard-fail on HW.
    k_d = nc.dram_tensor("k", [H, 128], f32, kind="ExternalInput")
    q_d = nc.dram_tensor("q", [H, 128], f32, kind="ExternalInput")
    id_d = nc.dram_tensor("ident", [128, 128], f32, kind="ExternalInput")
    bd_d = nc.dram_tensor("bdiag", [128, 128], f32, kind="ExternalInput")
    w_d = nc.dram_tensor("w", [n_loc, L], f32, kind="ExternalOutput")

    ngroups = n_loc // GROUP_TOK

    with tile.TileContext(nc) as tc:
        with (
            tc.tile_pool(name="consts", bufs=1) as consts,
            tc.tile_pool(name="ld", bufs=3) as ldp,
            tc.tile_pool(name="yT", bufs=4) as yTp,
            tc.tile_pool(name="pkq", bufs=4) as pkqp,
            tc.tile_pool(name="soft", bufs=3) as softp,
            tc.tile_pool(name="wout", bufs=3) as woutp,
            tc.tile_pool(name="stats", bufs=6) as statp,
            tc.tile_pool(name="ps_t", bufs=2, space="PSUM") as ps_t,
            tc.tile_pool(name="ps_p", bufs=2, space="PSUM") as ps_p,
            tc.tile_pool(name="ps_m", bufs=2, space="PSUM") as ps_m,
            tc.tile_pool(name="ps_c", bufs=2, space="PSUM") as ps_c,
        ):
            k_sb = consts.tile([128, 2, 128], f32)
            q_sb = consts.tile([128, 2, 128], f32)
            id_sb = consts.tile([128, 128], f32)
            bd_sb = consts.tile([128, 128], f32)
            nc.sync.dma_start(out=k_sb[:], in_=k_d.rearrange("(c p) d -> p c d", p=128))
            nc.sync.dma_start(out=q_sb[:], in_=q_d.rearrange("(c p) d -> p c d", p=128))
            nc.sync.dma_start(out=id_sb[:], in_=id_d[:])
            nc.sync.dma_start(out=bd_sb[:], in_=bd_d[:])

            for g in range(ngroups):
                row0 = g * GROUP_TOK
                y_sb = ldp.tile([128, GROUP_TILES, H], f32, tag="y")
                yh_sb = ldp.tile([128, GROUP_TILES, H], f32, tag="yh")
                nc.sync.dma_start(
                    out=y_sb[:],
                    in_=y_d[row0 : row0 + GROUP_TOK, :].rearrange(
                        "(t p) h -> p t h", p=128
                    ),
                )
                nc.sync.dma_start(
                    out=yh_sb[:],
                    in_=yh_d[row0 : row0 + GROUP_TOK, :].rearrange(
                        "(t p) h -> p t h", p=128
                    ),
                )

                ykq_sbs = []
                for t in range(GROUP_TILES):
                    # transpose both H-chunks of y and yhat: [tok,H] -> [H,tok]
                    yT_ps = ps_t.tile([128, 4, TILE_TOK], f32)
                    nc.tensor.transpose(yT_ps[:, 0, :], y_sb[:, t, 0:128], id_sb[:])
                    nc.tensor.transpose(yT_ps[:, 1, :], y_sb[:, t, 128:256], id_sb[:])
                    nc.tensor.transpose(yT_ps[:, 2, :], yh_sb[:, t, 0:128], id_sb[:])
                    nc.tensor.transpose(yT_ps[:, 3, :], yh_sb[:, t, 128:256], id_sb[:])
                    yT_sb = yTp.tile([128, 4, TILE_TOK], f32)
                    nc.vector.tensor_copy(yT_sb[:, 0:2, :], yT_ps[:, 0:2, :])
                    nc.scalar.copy(yT_sb[:, 2:4, :], yT_ps[:, 2:4, :])
                    # projections: ykT = k_pad.T @ yT, yqT = q_pad.T @ yT
                    # out is [128, tok] with partitions 64..127 all zero
                    ykq_ps = ps_p.tile([128, 2, TILE_TOK], f32)
                    for c in range(2):
                        nc.tensor.matmul(
                            ykq_ps[:, 0, :],
                            k_sb[:, c, :],
                            yT_sb[:, c, :],
                            start=(c == 0),
                            stop=(c == 1),
                        )
                    for c in range(2):
                        nc.tensor.matmul(
                            ykq_ps[:, 1, :],
                            q_sb[:, c, :],
                            yT_sb[:, 2 + c, :],
                            start=(c == 0),
                            stop=(c == 1),
                        )
                    ykq_sb = pkqp.tile([128, 2, TILE_TOK], f32)
                    if t % 2 == 0:
                        nc.vector.tensor_copy(ykq_sb[:], ykq_ps[:])
                    else:
                        nc.scalar.copy(ykq_sb[:], ykq_ps[:])
                    ykq_sbs.append(ykq_sb)

                # segment matmuls: M[l, m] = sum_d yk[l, d] * yq[m, d]
                # K=128 with zero upper halves -- exact, all row-base-0
                M_ps = ps_m.tile([128, GROUP_TILES, L], f32)
                for t in range(GROUP_TILES):
                    ykq_sb = ykq_sbs[t]
                    for s in range(2):
                        nc.tensor.matmul(
                            M_ps[s * 64 : (s + 1) * 64, t, :],
                            ykq_sb[:, 0, s * 64 : (s + 1) * 64],
                            ykq_sb[:, 1, s * 64 : (s + 1) * 64],
                            start=True,
                            stop=True,
                        )

                # row softmax over the free axis (m)
                maxv = statp.tile([128, GROUP_TILES], f32, tag="maxv")
                nbias = statp.tile([128, GROUP_TILES], f32, tag="nbias")
                rowsum = statp.tile([128, GROUP_TILES], f32, tag="rowsum")
                rr = statp.tile([128, GROUP_TILES], f32, tag="rr")
                nc.vector.tensor_reduce(
                    maxv[:], M_ps[:], axis=mybir.AxisListType.X, op=OP.max
                )
                nc.vector.tensor_scalar_mul(nbias[:], maxv[:], -SCALE)
                A_sb = softp.tile([128, GROUP_TILES, L], f32, tag="A")
                for b in range(GROUP_TILES):
                    nc.scalar.activation(
                        A_sb[:, b, :],
                        M_ps[:, b, :],
                        FT.Exp,
                        bias=nbias[:, b : b + 1],
                        scale=SCALE,
                        accum_out=rowsum[:, b : b + 1],
                    )
                nc.vector.reciprocal(rr[:], rowsum[:])
                # attn' = E * (1/rowsum) + EPS
                for b in range(GROUP_TILES):
                    nc.gpsimd.tensor_scalar(
                        A_sb[:, b, :],
                        A_sb[:, b, :],
                        rr[:, b : b + 1],
                        EPS,
                        op0=OP.mult,
                        op1=OP.add,
                    )
                # per-segment column sums, broadcast to all 64 rows:
                # CS[p, n] = sum_{K in seg(p)} attn'[K, n]
                CS_ps = ps_c.tile([128, GROUP_TILES, L], f32)
                nc.tensor.matmul(CS_ps[:], bd_sb[:], A_sb[:], start=True, stop=True)
                # rc = 1/CS via exp(-ln(CS)) on ACT (avoids slow DVE divide)
                lncs = softp.tile([128, GROUP_TILES, L], f32, tag="lncs")
                rc_sb = softp.tile([128, GROUP_TILES, L], f32, tag="rc")
                nc.scalar.activation(lncs[:], CS_ps[:], FT.Ln)
                nc.scalar.activation(rc_sb[:], lncs[:], FT.Exp, scale=-1.0)
                W_sb = woutp.tile([128, GROUP_TILES, L], f32)
                nc.gpsimd.tensor_tensor(W_sb[:], A_sb[:], rc_sb[:], op=OP.mult)
                nc.sync.dma_start(
                    out=w_d[row0 : row0 + GROUP_TOK, :].rearrange(
                        "(t p) m -> p t m", p=128
                    ),
                    in_=W_sb[:],
                )

    nc.compile()
    return nc


def _consts():
    ident = np.eye(128, dtype=np.float32)
    bdiag = np.zeros((128, 128), dtype=np.float32)
    bdiag[:64, :64] = 1.0
    bdiag[64:, 64:] = 1.0
    return ident, bdiag


def _pad_proj(m):
    """[H, D] -> [H, 128] with zero right half."""
    out = np.zeros((H, 128), dtype=np.float32)
    out[:, :D] = np.asarray(m, dtype=np.float32)
    return out


def _get_program(n_loc):
    if n_loc not in _CACHE:
        _CACHE[n_loc] = _build_program(n_loc)
    return _CACHE[n_loc]


def _prepare(yhat_embedding, y_embedding, k, q):
    nc = _get_program(N_LOC)
    ident, bdiag = _consts()
    y = np.ascontiguousarray(np.asarray(y_embedding, dtype=np.float32))
    yh = np.ascontiguousarray(np.asarray(yhat_embedding, dtype=np.float32))
    kk = _pad_proj(k)
    qq = _pad_proj(q)
    in_maps = []
    for i in range(NCORES):
        sl = slice(i * N_LOC, (i + 1) * N_LOC)
        in_maps.append(
            {
                "y": y[sl],
                "yh": yh[sl],
                "k": kk,
                "q": qq,
                "ident": ident,
                "bdiag": bdiag,
            }
        )
    return nc, in_maps


def _run(yhat_embedding, y_embedding, k, q, trace=False):
    from concourse.bass_utils import run_bass_kernel_spmd

    nc, in_maps = _prepare(yhat_embedding, y_embedding, k, q)
    res = run_bass_kernel_spmd(
        nc, in_maps, core_ids=list(range(NCORES)), trace=trace
    )
    w = np.concatenate([r["w"] for r in res.results], axis=0)
    out = w.reshape(N_TOTAL // L, L, L)
    return out, res


def kernel(**inputs):
    yhat_embedding = inputs["yhat_embedding"]
    y_embedding = inputs["y_embedding"]
    k = inputs["k"]
    q = inputs["q"]
    seg_len = int(inputs.get("seg_len", L))
    assert seg_len == L, f"kernel hardcodes seg_len={L}, got {seg_len}"
    out, _ = _run(yhat_embedding, y_embedding, k, q, trace=False)
    return out

